# revision 27
# baseline (speedup 1.0000x reference)
"""PowerAttention fused kernel for 8 Trainium2 NeuronCores.

Model (reference): B=1, T=2048, C=2048, H=8 kv heads, R=2 (16 q heads),
D=128, DEG=2:
  q,k,v,g projections -> gated power attention (qk^2 with cumulative
  log-sigmoid gate decay, causal) -> output projection.

Sharding: head-parallel. Core i owns kv head i and query heads {2i, 2i+1}
(column slices of Wq/Wk/Wv/Wg). Attention is entirely local per core. The
y-activations are AllGathered in four t-chunks (512KB/rank each) so the
collectives overlap attention of later chunks; the output projection is
column-sharded (core i computes out[:, i*256:(i+1)*256] from its Wc
column slice, emitted transposed; the host transposes and concatenates).

Key algebraic trick: the gate factor exp(logG_t - logG_s) factors into
exp(-cs_t/2)/sqrt(D) folded into q (free-dim broadcast multiply) and
exp(+cs_s/2) applied as the per-partition scale of the squaring
activation, where cs = cumsum(softplus(-off - g)) = -logG. The cumsum is
computed with small triangular matmuls. All large matmuls run as
float32r (full-rate fp32 on the PE array).
"""
import ml_dtypes
import numpy as np
from contextlib import ExitStack

import concourse.bass as bass
import concourse.bacc as bacc
import concourse.tile as tile
import concourse.mybir as mybir
from concourse.bass_utils import run_bass_kernel_spmd

F32 = mybir.dt.float32
F32R = mybir.dt.float32r
BF16 = mybir.dt.bfloat16
AF = mybir.ActivationFunctionType

N_CORES = 8
H = 8
R = 2
D = 128
T = 2048
C = 2048
GATE_OFFSET = 6.906768
NEG_HALF_LOG_D = -0.5 * float(np.log(D))  # fold 1/sqrt(D) into the q gate

NT = T // 512    # 4 free-dim chunks of 512
NC16 = C // 128  # 16 contraction chunks

_CACHE = {}


def _build():
    nc = bacc.Bacc("TRN2", target_bir_lowering=False, debug=False,
                   num_devices=N_CORES)

    # register float activation-bias constants (bias floats lower to const
    # APs for non-Copy activation functions)
    for cidx, cval in enumerate((-GATE_OFFSET, NEG_HALF_LOG_D)):
        cten = nc.alloc_sbuf_tensor(f"constx_{cidx}", [128, 1], F32)
        nc.gpsimd.memset(cten.ap(), cval)
        nc.const_aps.aps[(F32, cval)] = cten.ap()
    nc.all_engine_barrier()

    hst = nc.dram_tensor("hst", [NC16 * NT, 128, 512], F32R,
                     kind="ExternalInput").ap()
    wq = nc.dram_tensor("wq", [C, 2 * D], F32R, kind="ExternalInput").ap()
    wk = nc.dram_tensor("wk", [C, D], F32R, kind="ExternalInput").ap()
    wv = nc.dram_tensor("wv", [C, D], F32R, kind="ExternalInput").ap()
    wg = nc.dram_tensor("wg", [C, 1], F32R, kind="ExternalInput").ap()
    wcs = nc.dram_tensor("wcs", [C, 2 * D], BF16, kind="ExternalInput").ap()
    bcs = nc.dram_tensor("bcs", [128, 2], F32, kind="ExternalInput").ap()
    tri_in = nc.dram_tensor("tri", [128, 128], F32R, kind="ExternalInput").ap()
    tri16_in = nc.dram_tensor("tri16", [16, 16], F32R,
                              kind="ExternalInput").ap()
    ident_in = nc.dram_tensor("ident", [128, 128], F32R,
                              kind="ExternalInput").ap()
    onesc_in = nc.dram_tensor("onesc", [128, 1], F32R,
                              kind="ExternalInput").ap()
    onesr_in = nc.dram_tensor("onesr", [1, 128], F32R,
                              kind="ExternalInput").ap()
    outT = nc.dram_tensor("outT", [2 * D, T], F32, kind="ExternalOutput").ap()

    y_send = [nc.dram_tensor(f"y_send{c}", [2 * D, 512], BF16)
              for c in range(NT)]
    y_all = [nc.dram_tensor(f"y_all{c}", [16 * D, 512], BF16,
                            addr_space="Shared") for c in range(NT)]
    warm_in = nc.dram_tensor("warm_in", [8, 32], F32)
    warm_out = nc.dram_tensor("warm_out", [64, 32], F32)

    with tile.TileContext(nc) as tc, ExitStack() as ctx:
        const = ctx.enter_context(tc.tile_pool(name="const", bufs=1))
        big = ctx.enter_context(tc.tile_pool(name="big", bufs=1))
        work = ctx.enter_context(tc.tile_pool(name="work", bufs=3))

        # ---- constants ----
        tri = const.tile([128, 128], F32R, tag="tri")
        nc.sync.dma_start(tri[:], tri_in[:])
        tri16 = const.tile([16, 16], F32R, tag="tri16")
        nc.sync.dma_start(tri16[:], tri16_in[:])
        ident = const.tile([128, 128], F32R, tag="ident")
        nc.sync.dma_start(ident[:], ident_in[:])
        ones_col = const.tile([128, 1], F32R, tag="onesc")
        nc.sync.dma_start(ones_col[:], onesc_in[:])
        ones_row = const.tile([1, 128], F32R, tag="onesr")
        nc.sync.dma_start(ones_row[:], onesr_in[:])
        bc_sb = const.tile([128, 2], F32, tag="bcs")
        nc.sync.dma_start(bc_sb[:], bcs[:])
        wcs_sb = big.tile([128, NC16 * 256], BF16, tag="wcs")

        # ---- persistent activations ----
        qT = [big.tile([128, T], F32R, tag=f"qT{h}", name=f"qT{h}")
              for h in range(2)]
        kT = big.tile([128, T], F32R, tag="kT")
        vsb = big.tile([128, T], F32R, tag="V")      # [s-part, 16 x d']
        gbc = big.tile([128, T], F32R, tag="gbc")    # exp(-cs/2)/sqrt(D)
        yT = [big.tile([128, T], BF16, tag=f"yT{h}", name=f"yT{h}")
              for h in range(2)]
        sp_row = big.tile([1, T], F32, tag="sp")     # softplus(-off-g)
        ginv = big.tile([128, NC16], F32, tag="ginv")  # exp(+cs/2)

        # ===== Phase 1: q/k/v/g projections (transposed layouts) =====
        with tc.tile_pool(name="pw", bufs=1) as pw:
          with tc.tile_pool(name="pj", bufs=1, space="PSUM") as pj:
            wq_sb = pw.tile([128, NC16 * 256], F32R, tag="wq")
            wk_sb = pw.tile([128, NC16 * 128], F32R, tag="wk")
            wv_sb = pw.tile([128, NC16 * 128], F32R, tag="wv")
            wg_sb = pw.tile([128, NC16], F32R, tag="wg")
            vT = pw.tile([128, T], F32R, tag="vT")
            ps_lg = pj.tile([128, NC16], F32, tag="lg")

            for n in range(NT):
                ps_q0 = pj.tile([128, 512], F32, tag="q0")
                ps_q1 = pj.tile([128, 512], F32, tag="q1")
                ps_k = pj.tile([128, 512], F32, tag="k")
                ps_v = pj.tile([128, 512], F32, tag="v")
                ps_g = pj.tile([1, 512], F32, tag="g")
                for c in range(NC16):
                    if n == 0:
                        # just-in-time weight loads so the first matmuls
                        # do not sit behind the full weight transfer
                        nc.sync.dma_start(wq_sb[:, c * 256:(c + 1) * 256],
                                          wq[c * 128:(c + 1) * 128, :])
                        nc.sync.dma_start(wk_sb[:, c * 128:(c + 1) * 128],
                                          wk[c * 128:(c + 1) * 128, :])
                        nc.sync.dma_start(wv_sb[:, c * 128:(c + 1) * 128],
                                          wv[c * 128:(c + 1) * 128, :])
                        nc.sync.dma_start(wg_sb[:, c:c + 1],
                                          wg[c * 128:(c + 1) * 128, :])
                    hs_t = work.tile([128, 512], F32R, tag="hst", bufs=12)
                    dma_eng = (nc.sync, nc.gpsimd)[c % 2]
                    dma_eng.dma_start(hs_t[:], hst[c * NT + n])
                    st = dict(start=(c == 0), stop=(c == NC16 - 1))
                    nc.tensor.matmul(ps_q0[:],
                                     wq_sb[:, c * 256:c * 256 + 128],
                                     hs_t[:], **st)
                    nc.tensor.matmul(ps_q1[:],
                                     wq_sb[:, c * 256 + 128:(c + 1) * 256],
                                     hs_t[:], **st)
                    nc.tensor.matmul(ps_k[:], wk_sb[:, c * 128:(c + 1) * 128],
                                     hs_t[:], **st)
                    nc.tensor.matmul(ps_v[:], wv_sb[:, c * 128:(c + 1) * 128],
                                     hs_t[:], **st)
                    nc.tensor.matmul(ps_g[:], wg_sb[:, c:c + 1], hs_t[:],
                                     **st)
                sl = np.s_[:, n * 512:(n + 1) * 512]
                nc.vector.tensor_copy(qT[0][sl], ps_q0[:])
                nc.vector.tensor_copy(qT[1][sl], ps_q1[:])
                nc.vector.tensor_copy(kT[sl], ps_k[:])
                nc.vector.tensor_copy(vT[sl], ps_v[:])
                # softplus(-g - off) = ln(1 + exp(-g - off))  (no Softplus
                # LUT on this build; Exp/Ln/Square/Copy share one table)
                u_row = work.tile([1, 512], F32, tag="urow")
                nc.scalar.activation(u_row[:], ps_g[:], AF.Exp,
                                     scale=-1.0, bias=-GATE_OFFSET)
                nc.scalar.activation(sp_row[0:1, n * 512:(n + 1) * 512],
                                     u_row[:], AF.Ln, bias=1.0)
                # transpose this chunk's gate rows now (shortens the
                # serial gating chain after the projections)
                for k in range(4):
                    f = n * 4 + k
                    nc.tensor.transpose(ps_lg[:, f:f + 1],
                                        sp_row[0:1, f * 128:(f + 1) * 128],
                                        ident[0:1, 0:1].bitcast(F32))

          # out-proj weights: needed only from the first out-proj chunk
          for hd in range(NC16):
              nc.sync.dma_start(wcs_sb[:, hd * 256:(hd + 1) * 256],
                                wcs[hd * 128:(hd + 1) * 128, :])

          # ===== Phase 2: gate cumsum + broadcast factors =====
          with tc.tile_pool(name="pg", bufs=1, space="PSUM") as pg:
              sp_pf = work.tile([128, NC16], F32, tag="sppf")
              nc.vector.tensor_copy(sp_pf[:], ps_lg[:])
              # in-block inclusive cumsum: cs[t,f] = sum_{s<=t} sp[s,f]
              ps_cs = pg.tile([128, NC16], F32, tag="cs")
              nc.tensor.matmul(ps_cs[:], tri[:].bitcast(F32), sp_pf[:],
                               start=True, stop=True)
              # block sums via ones-column matmul
              ps_bsr = pg.tile([1, NC16], F32, tag="small")
              nc.tensor.matmul(ps_bsr[:], ones_col[:].bitcast(F32),
                               sp_pf[:], start=True, stop=True)
              bs_row = work.tile([1, NC16], F32, tag="bsr")
              nc.vector.tensor_copy(bs_row[:], ps_bsr[:])
              ps_bs = pg.tile([16, 1], F32, tag="small")
              nc.tensor.transpose(ps_bs[:], bs_row[:],
                                  ident[0:1, 0:1].bitcast(F32))
              bs_col = work.tile([16, 1], F32, tag="bsc")
              nc.vector.tensor_copy(bs_col[:], ps_bs[:])
              # exclusive block prefix: ebp[f] = sum_{g<f} bs[g]
              ps_ebp = pg.tile([16, 1], F32, tag="small2")
              nc.tensor.matmul(ps_ebp[:], tri16[:].bitcast(F32),
                               bs_col[:], start=True, stop=True)
              ebp_col = work.tile([16, 1], F32, tag="ebpc")
              nc.vector.tensor_copy(ebp_col[:], ps_ebp[:])
              ps_er = pg.tile([1, 16], F32, tag="small")
              nc.tensor.transpose(ps_er[:], ebp_col[:],
                                  ident[0:16, 0:16].bitcast(F32))
              ebp_row = work.tile([1, 16], F32, tag="ebpr")
              nc.vector.tensor_copy(ebp_row[:], ps_er[:])
              ps_ebc = pg.tile([128, NC16], F32, tag="ebc")
              nc.tensor.matmul(ps_ebc[:], ones_row[:].bitcast(F32),
                               ebp_row[:], start=True, stop=True)
              cs_sb = work.tile([128, NC16], F32, tag="cssb")
              nc.vector.tensor_copy(cs_sb[:], ps_cs[:])
              cs_tot = big.tile([128, NC16], F32, tag="cstot")
              nc.vector.tensor_add(cs_tot[:], cs_sb[:], ps_ebc[:])
              # per-partition gate scale for the squaring step
              nc.scalar.activation(ginv[:], cs_tot[:], AF.Exp, scale=0.5)
              # row layout for the q-side factor
              for tn in range(NT):
                  ps_csr = pg.tile([1, 512], F32, tag="csr")
                  for k in range(4):
                      f = tn * 4 + k
                      nc.tensor.transpose(ps_csr[0:1, k * 128:(k + 1) * 128],
                                          cs_tot[:, f:f + 1],
                                          ident[:].bitcast(F32))
                  gt_row = work.tile([1, 512], F32R, tag="gtr")
                  nc.scalar.activation(gt_row[:], ps_csr[:], AF.Exp,
                                       scale=-0.5, bias=NEG_HALF_LOG_D)
                  ps_gbc = pg.tile([128, 512], F32, tag="gbcp")
                  nc.tensor.matmul(ps_gbc[:], ones_row[:], gt_row[:],
                                   start=True, stop=True)
                  nc.vector.tensor_copy(gbc[:, tn * 512:(tn + 1) * 512],
                                        ps_gbc[:])
              # scale q by gate factor (in place)
              for h in range(2):
                  nc.vector.tensor_mul(qT[h][:], qT[h][:], gbc[:])
              # transpose V^T -> V [s-part, d'] blocks
              for j in range(NC16):
                  ps_vt = pg.tile([128, 128], F32R, tag="vtr")
                  nc.tensor.transpose(ps_vt[:],
                                      vT[:, j * 128:(j + 1) * 128], ident[:])
                  nc.vector.tensor_copy(vsb[:, j * 128:(j + 1) * 128],
                                        ps_vt[:])

        # warm up the collective stream early so the first real
        # AllGather does not pay first-op setup costs
        nc.gpsimd.collective_compute(
            "AllGather", mybir.AluOpType.bypass,
            ins=[warm_in[:]], outs=[warm_out[:]],
            replica_groups=[list(range(N_CORES))],
        )

        # ===== Phase 3+4+5: attention chunks, chunked AllGather, and
        # output projection pipelined per t-chunk =====
        with tc.tile_pool(name="at", bufs=20) as atp, \
                tc.tile_pool(name="pa", bufs=1, space="PSUM") as pa, \
                tc.tile_pool(name="po", bufs=1, space="PSUM") as po:
            for cch in range(NT):
                tsl = np.s_[cch * 512:(cch + 1) * 512]
                jmax = 4 * cch + 4
                for h in range(2):
                    ps_y = pa.tile([128, 512], F32, tag="y", bufs=2)
                    ps_s0 = pa.tile([1, 512], F32, tag="s0")
                    ats = []
                    for j in range(jmax):
                        off = max(0, j * 128 - cch * 512)
                        ps_s = pa.tile([128, 512], F32, tag="s", bufs=3)
                        nc.tensor.matmul(
                            ps_s[:, off:512], kT[:, j * 128:(j + 1) * 128],
                            qT[h][:, cch * 512 + off:(cch + 1) * 512],
                            start=True, stop=True)
                        at = atp.tile([128, 512], F32R, tag="at")
                        nc.scalar.activation(at[:, off:512],
                                             ps_s[:, off:512], AF.Square,
                                             scale=ginv[:, j:j + 1])
                        if j * 128 >= cch * 512:  # diagonal: causal mask
                            nc.vector.tensor_mul(at[:, off:off + 128],
                                                 at[:, off:off + 128],
                                                 tri[:])
                        ats.append((at, off))
                    for j, (at, off) in enumerate(ats):
                        nc.tensor.matmul(ps_y[:, off:512],
                                         vsb[:, j * 128:(j + 1) * 128],
                                         at[:, off:512],
                                         start=(j == 0), stop=(j == jmax - 1))
                    for j, (at, off) in enumerate(ats):
                        nc.tensor.matmul(ps_s0[:, off:512], ones_col[:],
                                         at[:, off:512],
                                         start=(j == 0), stop=(j == jmax - 1))
                    s0e = work.tile([1, 512], F32R, tag="s0e")
                    nc.scalar.activation(s0e[:], ps_s0[:], AF.Copy, bias=1e-6)
                    ps_rb = pa.tile([128, 512], F32, tag="s", bufs=3)
                    nc.tensor.matmul(ps_rb[:], ones_row[:], s0e[:],
                                     start=True, stop=True)
                    rb = work.tile([128, 512], F32, tag="rb2")
                    nc.vector.reciprocal(rb[:], ps_rb[:])
                    nc.vector.tensor_mul(yT[h][:, tsl], ps_y[:], rb[:])
                    # stage into the AllGather send buffer on the scalar
                    # trigger queue so it never sits behind yrow traffic
                    nc.scalar.dma_start(
                        y_send[cch][h * 128:(h + 1) * 128, :], yT[h][:, tsl])
                # chunk AllGather (overlaps later chunks' attention)
                nc.gpsimd.collective_compute(
                    "AllGather",
                    mybir.AluOpType.bypass,
                    ins=[y_send[cch][:]],
                    outs=[y_all[cch][:]],
                    replica_groups=[list(range(N_CORES))],
                )


            # out-proj for all t-chunks, after attention in program order
            # so chunk c+1 attention never queues behind chunk-c out-proj
            for cch in range(NT):
                tsl = np.s_[cch * 512:(cch + 1) * 512]
                ps_o = [po.tile([128, 512], F32, tag=f"o{ct}",
                                name=f"o{ct}_{cch}") for ct in range(2)]
                for hd in range(NC16):
                    yrow = work.tile([128, 512], BF16, tag="yrow", bufs=6)
                    yq = nc.gpsimd if (cch == NT - 1 and hd % 2) else nc.sync
                    yq.dma_start(yrow[:],
                                 y_all[cch][hd * 128:(hd + 1) * 128, :])
                    for ct in range(2):
                        nc.tensor.matmul(
                            ps_o[ct][:],
                            wcs_sb[:, hd * 256 + ct * 128:
                                   hd * 256 + (ct + 1) * 128],
                            yrow[:], start=(hd == 0), stop=(hd == NC16 - 1))
                for ct in range(2):
                    osb = work.tile([128, 512], F32, tag="osb")
                    nc.vector.tensor_scalar_add(osb[:], ps_o[ct][:],
                                                bc_sb[:, ct:ct + 1])
                    nc.sync.dma_start(outT[ct * 128:(ct + 1) * 128, tsl],
                                      osb[:])

    nc.compile()
    return nc


def _get_nc():
    if "nc" not in _CACHE:
        _CACHE["nc"] = _build()
    return _CACHE["nc"]


def _make_in_maps(hidden_states, Wq, Wk, Wv, Wg, Wc, bc):
    hsT = np.ascontiguousarray(hidden_states.reshape(T, C).T,
                               dtype=np.float32)
    # pre-tile to [c*NT+n, 128, 512] so each projection DMA is one
    # contiguous 256KB burst
    hsT = np.ascontiguousarray(
        hsT.reshape(NC16, 128, NT, 512).transpose(0, 2, 1, 3)
        .reshape(NC16 * NT, 128, 512))
    tri = np.triu(np.ones((128, 128), dtype=np.float32))
    tri16 = np.triu(np.ones((16, 16), dtype=np.float32), k=1)
    ident = np.eye(128, dtype=np.float32)
    onesc = np.ones((128, 1), dtype=np.float32)
    onesr = np.ones((1, 128), dtype=np.float32)
    in_maps = []
    for i in range(N_CORES):
        in_maps.append({
            "hst": hsT,
            "wq": np.ascontiguousarray(
                Wq[:, i * 256:(i + 1) * 256], dtype=np.float32),
            "wk": np.ascontiguousarray(
                Wk[:, i * 128:(i + 1) * 128], dtype=np.float32),
            "wv": np.ascontiguousarray(
                Wv[:, i * 128:(i + 1) * 128], dtype=np.float32),
            "wg": np.ascontiguousarray(Wg[:, i:i + 1], dtype=np.float32),
            "wcs": np.ascontiguousarray(
                Wc[:, i * 256:(i + 1) * 256]).astype(ml_dtypes.bfloat16),
            "bcs": np.ascontiguousarray(
                bc[i * 256:(i + 1) * 256].reshape(2, 128).T,
                dtype=np.float32),
            "tri": tri,
            "tri16": tri16,
            "ident": ident,
            "onesc": onesc,
            "onesr": onesr,
        })
    return in_maps


def _run(in_maps, trace=False):
    nc = _get_nc()
    kw = {"tmpdir": "/tmp/trace_out"} if trace else {}
    res = run_bass_kernel_spmd(nc, in_maps, list(range(N_CORES)),
                               trace=trace, **kw)
    out = np.empty((T, C), dtype=np.float32)
    for i in range(N_CORES):
        out[:, i * 256:(i + 1) * 256] = res.results[i]["outT"].T
    return out.reshape(1, T, C), res


def kernel(hidden_states, Wq, Wk, Wv, Wg, Wc, bc):
    in_maps = _make_in_maps(np.asarray(hidden_states), np.asarray(Wq),
                            np.asarray(Wk), np.asarray(Wv), np.asarray(Wg),
                            np.asarray(Wc), np.asarray(bc))
    out, _ = _run(in_maps)
    return out


# revision 28
# speedup vs baseline: 1.0502x; 1.0502x over previous
"""PowerAttention fused kernel for 8 Trainium2 NeuronCores.

Model (reference): B=1, T=2048, C=2048, H=8 kv heads, R=2 (16 q heads),
D=128, DEG=2:
  q,k,v,g projections -> gated power attention (qk^2 with cumulative
  log-sigmoid gate decay, causal) -> output projection.

Sharding: head-parallel. Core i owns kv head i and query heads {2i, 2i+1}
(column slices of Wq/Wk/Wv/Wg). Attention is entirely local per core. The
y-activations are AllGathered in four t-chunks (512KB/rank each) so the
collectives overlap attention of later chunks; the output projection is
column-sharded (core i computes out[:, i*256:(i+1)*256] from its Wc
column slice, emitted transposed; the host transposes and concatenates).

Key algebraic trick: the gate factor exp(logG_t - logG_s) factors into
exp(-cs_t/2)/sqrt(D) folded into q (free-dim broadcast multiply) and
exp(+cs_s/2) applied as the per-partition scale of the squaring
activation, where cs = cumsum(softplus(-off - g)) = -logG. The cumsum is
computed with small triangular matmuls. All large matmuls run as
float32r (full-rate fp32 on the PE array).
"""
import ml_dtypes
import numpy as np
from contextlib import ExitStack

import concourse.bass as bass
import concourse.bacc as bacc
import concourse.tile as tile
import concourse.mybir as mybir
from concourse.bass_utils import run_bass_kernel_spmd

F32 = mybir.dt.float32
F32R = mybir.dt.float32r
BF16 = mybir.dt.bfloat16
AF = mybir.ActivationFunctionType

N_CORES = 8
H = 8
R = 2
D = 128
T = 2048
C = 2048
GATE_OFFSET = 6.906768
NEG_HALF_LOG_D = -0.5 * float(np.log(D))  # fold 1/sqrt(D) into the q gate

NT = T // 512    # 4 free-dim chunks of 512
NC16 = C // 128  # 16 contraction chunks

_CACHE = {}


def _build():
    nc = bacc.Bacc("TRN2", target_bir_lowering=False, debug=False,
                   num_devices=N_CORES)

    # register float activation-bias constants (bias floats lower to const
    # APs for non-Copy activation functions)
    for cidx, cval in enumerate((-GATE_OFFSET, NEG_HALF_LOG_D)):
        cten = nc.alloc_sbuf_tensor(f"constx_{cidx}", [128, 1], F32)
        nc.gpsimd.memset(cten.ap(), cval)
        nc.const_aps.aps[(F32, cval)] = cten.ap()
    nc.all_engine_barrier()

    hst = nc.dram_tensor("hst", [NC16 * NT, 128, 512], F32R,
                     kind="ExternalInput").ap()
    wq = nc.dram_tensor("wq", [C, 2 * D], F32R, kind="ExternalInput").ap()
    wk = nc.dram_tensor("wk", [C, D], F32R, kind="ExternalInput").ap()
    wv = nc.dram_tensor("wv", [C, D], F32R, kind="ExternalInput").ap()
    wg = nc.dram_tensor("wg", [C, 1], F32R, kind="ExternalInput").ap()
    wcs = nc.dram_tensor("wcs", [C, 2 * D], BF16, kind="ExternalInput").ap()
    bcs = nc.dram_tensor("bcs", [128, 2], F32, kind="ExternalInput").ap()
    tri_in = nc.dram_tensor("tri", [128, 128], F32R, kind="ExternalInput").ap()
    tri16_in = nc.dram_tensor("tri16", [16, 16], F32R,
                              kind="ExternalInput").ap()
    ident_in = nc.dram_tensor("ident", [128, 128], F32R,
                              kind="ExternalInput").ap()
    onesc_in = nc.dram_tensor("onesc", [128, 1], F32R,
                              kind="ExternalInput").ap()
    onesr_in = nc.dram_tensor("onesr", [1, 128], F32R,
                              kind="ExternalInput").ap()
    outT = nc.dram_tensor("outT", [2 * D, T], F32, kind="ExternalOutput").ap()

    y_send = [nc.dram_tensor(f"y_send{c}", [2 * D, 512], BF16)
              for c in range(NT)]
    y_all = [nc.dram_tensor(f"y_all{c}", [16 * D, 512], BF16,
                            addr_space="Shared") for c in range(NT)]
    warm_in = nc.dram_tensor("warm_in", [8, 32], F32)
    warm_out = nc.dram_tensor("warm_out", [64, 32], F32)

    with tile.TileContext(nc) as tc, ExitStack() as ctx:
        const = ctx.enter_context(tc.tile_pool(name="const", bufs=1))
        big = ctx.enter_context(tc.tile_pool(name="big", bufs=1))
        work = ctx.enter_context(tc.tile_pool(name="work", bufs=3))

        # ---- constants ----
        tri = const.tile([128, 128], F32R, tag="tri")
        nc.sync.dma_start(tri[:], tri_in[:])
        tri16 = const.tile([16, 16], F32R, tag="tri16")
        nc.sync.dma_start(tri16[:], tri16_in[:])
        ident = const.tile([128, 128], F32R, tag="ident")
        nc.sync.dma_start(ident[:], ident_in[:])
        ones_col = const.tile([128, 1], F32R, tag="onesc")
        nc.sync.dma_start(ones_col[:], onesc_in[:])
        ones_row = const.tile([1, 128], F32R, tag="onesr")
        nc.sync.dma_start(ones_row[:], onesr_in[:])
        bc_sb = const.tile([128, 2], F32, tag="bcs")
        nc.sync.dma_start(bc_sb[:], bcs[:])
        wcs_sb = big.tile([128, NC16 * 256], BF16, tag="wcs")

        # ---- persistent activations ----
        qT = [big.tile([128, T], F32R, tag=f"qT{h}", name=f"qT{h}")
              for h in range(2)]
        kT = big.tile([128, T], F32R, tag="kT")
        vsb = big.tile([128, T], F32R, tag="V")      # [s-part, 16 x d']
        gbc = big.tile([128, T], F32R, tag="gbc")    # exp(-cs/2)/sqrt(D)
        yT = [big.tile([128, T], BF16, tag=f"yT{h}", name=f"yT{h}")
              for h in range(2)]
        sp_row = big.tile([1, T], F32, tag="sp")     # softplus(-off-g)
        ginv = big.tile([128, NC16], F32, tag="ginv")  # exp(+cs/2)

        # ===== Phase 1: q/k/v/g projections (transposed layouts) =====
        with tc.tile_pool(name="pw", bufs=1) as pw:
          with tc.tile_pool(name="pj", bufs=1, space="PSUM") as pj:
            wq_sb = pw.tile([128, NC16 * 256], F32R, tag="wq")
            wk_sb = pw.tile([128, NC16 * 128], F32R, tag="wk")
            wv_sb = pw.tile([128, NC16 * 128], F32R, tag="wv")
            wg_sb = pw.tile([128, NC16], F32R, tag="wg")
            vT = pw.tile([128, T], F32R, tag="vT")
            ps_lg = pj.tile([128, NC16], F32, tag="lg")

            for n in range(NT):
                ps_q0 = pj.tile([128, 512], F32, tag="q0")
                ps_q1 = pj.tile([128, 512], F32, tag="q1")
                ps_k = pj.tile([128, 512], F32, tag="k")
                ps_v = pj.tile([128, 512], F32, tag="v")
                ps_g = pj.tile([1, 512], F32, tag="g")
                for c in range(NC16):
                    if n == 0:
                        # just-in-time weight loads so the first matmuls
                        # do not sit behind the full weight transfer
                        nc.sync.dma_start(wq_sb[:, c * 256:(c + 1) * 256],
                                          wq[c * 128:(c + 1) * 128, :])
                        nc.sync.dma_start(wk_sb[:, c * 128:(c + 1) * 128],
                                          wk[c * 128:(c + 1) * 128, :])
                        nc.sync.dma_start(wv_sb[:, c * 128:(c + 1) * 128],
                                          wv[c * 128:(c + 1) * 128, :])
                        nc.sync.dma_start(wg_sb[:, c:c + 1],
                                          wg[c * 128:(c + 1) * 128, :])
                    hs_t = work.tile([128, 512], F32R, tag="hst", bufs=12)
                    dma_eng = (nc.sync, nc.gpsimd)[c % 2]
                    dma_eng.dma_start(hs_t[:], hst[c * NT + n])
                    st = dict(start=(c == 0), stop=(c == NC16 - 1))
                    nc.tensor.matmul(ps_q0[:],
                                     wq_sb[:, c * 256:c * 256 + 128],
                                     hs_t[:], **st)
                    nc.tensor.matmul(ps_q1[:],
                                     wq_sb[:, c * 256 + 128:(c + 1) * 256],
                                     hs_t[:], **st)
                    nc.tensor.matmul(ps_k[:], wk_sb[:, c * 128:(c + 1) * 128],
                                     hs_t[:], **st)
                    nc.tensor.matmul(ps_v[:], wv_sb[:, c * 128:(c + 1) * 128],
                                     hs_t[:], **st)
                    nc.tensor.matmul(ps_g[:], wg_sb[:, c:c + 1], hs_t[:],
                                     **st)
                sl = np.s_[:, n * 512:(n + 1) * 512]
                nc.vector.tensor_copy(qT[0][sl], ps_q0[:])
                nc.vector.tensor_copy(qT[1][sl], ps_q1[:])
                nc.vector.tensor_copy(kT[sl], ps_k[:])
                nc.vector.tensor_copy(vT[sl], ps_v[:])
                # softplus(-g - off) = ln(1 + exp(-g - off))  (no Softplus
                # LUT on this build; Exp/Ln/Square/Copy share one table)
                u_row = work.tile([1, 512], F32, tag="urow")
                nc.scalar.activation(u_row[:], ps_g[:], AF.Exp,
                                     scale=-1.0, bias=-GATE_OFFSET)
                nc.scalar.activation(sp_row[0:1, n * 512:(n + 1) * 512],
                                     u_row[:], AF.Ln, bias=1.0)
                # transpose this chunk's gate rows now (shortens the
                # serial gating chain after the projections)
                for k in range(4):
                    f = n * 4 + k
                    nc.tensor.transpose(ps_lg[:, f:f + 1],
                                        sp_row[0:1, f * 128:(f + 1) * 128],
                                        ident[0:1, 0:1].bitcast(F32))

          # out-proj weights: needed only from the first out-proj chunk
          for hd in range(NC16):
              nc.sync.dma_start(wcs_sb[:, hd * 256:(hd + 1) * 256],
                                wcs[hd * 128:(hd + 1) * 128, :])

          # ===== Phase 2: gate cumsum + broadcast factors =====
          with tc.tile_pool(name="pg", bufs=1, space="PSUM") as pg:
              sp_pf = work.tile([128, NC16], F32, tag="sppf")
              nc.vector.tensor_copy(sp_pf[:], ps_lg[:])
              # in-block inclusive cumsum: cs[t,f] = sum_{s<=t} sp[s,f]
              ps_cs = pg.tile([128, NC16], F32, tag="cs")
              nc.tensor.matmul(ps_cs[:], tri[:].bitcast(F32), sp_pf[:],
                               start=True, stop=True)
              # block sums via ones-column matmul
              ps_bsr = pg.tile([1, NC16], F32, tag="small")
              nc.tensor.matmul(ps_bsr[:], ones_col[:].bitcast(F32),
                               sp_pf[:], start=True, stop=True)
              bs_row = work.tile([1, NC16], F32, tag="bsr")
              nc.vector.tensor_copy(bs_row[:], ps_bsr[:])
              ps_bs = pg.tile([16, 1], F32, tag="small")
              nc.tensor.transpose(ps_bs[:], bs_row[:],
                                  ident[0:1, 0:1].bitcast(F32))
              bs_col = work.tile([16, 1], F32, tag="bsc")
              nc.vector.tensor_copy(bs_col[:], ps_bs[:])
              # exclusive block prefix: ebp[f] = sum_{g<f} bs[g]
              ps_ebp = pg.tile([16, 1], F32, tag="small2")
              nc.tensor.matmul(ps_ebp[:], tri16[:].bitcast(F32),
                               bs_col[:], start=True, stop=True)
              ebp_col = work.tile([16, 1], F32, tag="ebpc")
              nc.vector.tensor_copy(ebp_col[:], ps_ebp[:])
              ps_er = pg.tile([1, 16], F32, tag="small")
              nc.tensor.transpose(ps_er[:], ebp_col[:],
                                  ident[0:16, 0:16].bitcast(F32))
              ebp_row = work.tile([1, 16], F32, tag="ebpr")
              nc.vector.tensor_copy(ebp_row[:], ps_er[:])
              ps_ebc = pg.tile([128, NC16], F32, tag="ebc")
              nc.tensor.matmul(ps_ebc[:], ones_row[:].bitcast(F32),
                               ebp_row[:], start=True, stop=True)
              cs_sb = work.tile([128, NC16], F32, tag="cssb")
              nc.vector.tensor_copy(cs_sb[:], ps_cs[:])
              cs_tot = big.tile([128, NC16], F32, tag="cstot")
              nc.vector.tensor_add(cs_tot[:], cs_sb[:], ps_ebc[:])
              # per-partition gate scale for the squaring step
              nc.scalar.activation(ginv[:], cs_tot[:], AF.Exp, scale=0.5)
              # row layout for the q-side factor
              for tn in range(NT):
                  ps_csr = pg.tile([1, 512], F32, tag="csr")
                  for k in range(4):
                      f = tn * 4 + k
                      nc.tensor.transpose(ps_csr[0:1, k * 128:(k + 1) * 128],
                                          cs_tot[:, f:f + 1],
                                          ident[:].bitcast(F32))
                  gt_row = work.tile([1, 512], F32R, tag="gtr")
                  nc.scalar.activation(gt_row[:], ps_csr[:], AF.Exp,
                                       scale=-0.5, bias=NEG_HALF_LOG_D)
                  ps_gbc = pg.tile([128, 512], F32, tag="gbcp")
                  nc.tensor.matmul(ps_gbc[:], ones_row[:], gt_row[:],
                                   start=True, stop=True)
                  nc.vector.tensor_copy(gbc[:, tn * 512:(tn + 1) * 512],
                                        ps_gbc[:])
              # scale q by gate factor (in place)
              for h in range(2):
                  nc.vector.tensor_mul(qT[h][:], qT[h][:], gbc[:])
              # transpose V^T -> V [s-part, d'] blocks
              for j in range(NC16):
                  ps_vt = pg.tile([128, 128], F32R, tag="vtr")
                  nc.tensor.transpose(ps_vt[:],
                                      vT[:, j * 128:(j + 1) * 128], ident[:])
                  nc.vector.tensor_copy(vsb[:, j * 128:(j + 1) * 128],
                                        ps_vt[:])

        # warm up the collective stream early so the first real
        # AllGather does not pay first-op setup costs
        nc.gpsimd.collective_compute(
            "AllGather", mybir.AluOpType.bypass,
            ins=[warm_in[:]], outs=[warm_out[:]],
            replica_groups=[list(range(N_CORES))],
        )

        # ===== Phase 3+4+5: attention chunks, chunked AllGather, and
        # output projection pipelined per t-chunk =====
        with tc.tile_pool(name="at", bufs=20) as atp, \
                tc.tile_pool(name="pa", bufs=1, space="PSUM") as pa, \
                tc.tile_pool(name="po", bufs=1, space="PSUM") as po:
            for cch in range(NT):
                tsl = np.s_[cch * 512:(cch + 1) * 512]
                jmax = 4 * cch + 4
                for h in range(2):
                    ps_y = pa.tile([128, 512], F32, tag="y", bufs=2)
                    ps_s0 = pa.tile([1, 512], F32, tag="s0")
                    ats = []
                    for j in range(jmax):
                        off = max(0, j * 128 - cch * 512)
                        ps_s = pa.tile([128, 512], F32, tag="s", bufs=3)
                        nc.tensor.matmul(
                            ps_s[:, off:512], kT[:, j * 128:(j + 1) * 128],
                            qT[h][:, cch * 512 + off:(cch + 1) * 512],
                            start=True, stop=True)
                        at = atp.tile([128, 512], F32R, tag="at")
                        nc.scalar.activation(at[:, off:512],
                                             ps_s[:, off:512], AF.Square,
                                             scale=ginv[:, j:j + 1])
                        if j * 128 >= cch * 512:  # diagonal: causal mask
                            nc.vector.tensor_mul(at[:, off:off + 128],
                                                 at[:, off:off + 128],
                                                 tri[:])
                        ats.append((at, off))
                    for j, (at, off) in enumerate(ats):
                        nc.tensor.matmul(ps_y[:, off:512],
                                         vsb[:, j * 128:(j + 1) * 128],
                                         at[:, off:512],
                                         start=(j == 0), stop=(j == jmax - 1))
                    for j, (at, off) in enumerate(ats):
                        nc.tensor.matmul(ps_s0[:, off:512], ones_col[:],
                                         at[:, off:512],
                                         start=(j == 0), stop=(j == jmax - 1))
                    s0e = work.tile([1, 512], F32R, tag="s0e")
                    nc.scalar.activation(s0e[:], ps_s0[:], AF.Copy, bias=1e-6)
                    ps_rb = pa.tile([128, 512], F32, tag="s", bufs=3)
                    nc.tensor.matmul(ps_rb[:], ones_row[:], s0e[:],
                                     start=True, stop=True)
                    rb = work.tile([128, 512], F32R, tag="rb2")
                    with nc.allow_low_precision(reason="f32r normalizer"):
                        nc.vector.reciprocal(rb[:], ps_rb[:])
                    yc = work.tile([128, 512], F32R, tag="yc")
                    nc.vector.tensor_copy(yc[:], ps_y[:])
                    nc.vector.tensor_mul(yT[h][:, tsl], yc[:], rb[:])
                    # stage into the AllGather send buffer on the scalar
                    # trigger queue so it never sits behind yrow traffic
                    nc.scalar.dma_start(
                        y_send[cch][h * 128:(h + 1) * 128, :], yT[h][:, tsl])
                # chunk AllGather (overlaps later chunks' attention)
                nc.gpsimd.collective_compute(
                    "AllGather",
                    mybir.AluOpType.bypass,
                    ins=[y_send[cch][:]],
                    outs=[y_all[cch][:]],
                    replica_groups=[list(range(N_CORES))],
                )


            # out-proj for all t-chunks, after attention in program order
            # so chunk c+1 attention never queues behind chunk-c out-proj
            for cch in range(NT):
                tsl = np.s_[cch * 512:(cch + 1) * 512]
                ps_o = [po.tile([128, 512], F32, tag=f"o{ct}",
                                name=f"o{ct}_{cch}") for ct in range(2)]
                for hd in range(NC16):
                    yrow = work.tile([128, 512], BF16, tag="yrow", bufs=6)
                    yq = nc.gpsimd if (cch == NT - 1 and hd % 2) else nc.sync
                    yq.dma_start(yrow[:],
                                 y_all[cch][hd * 128:(hd + 1) * 128, :])
                    for ct in range(2):
                        nc.tensor.matmul(
                            ps_o[ct][:],
                            wcs_sb[:, hd * 256 + ct * 128:
                                   hd * 256 + (ct + 1) * 128],
                            yrow[:], start=(hd == 0), stop=(hd == NC16 - 1))
                for ct in range(2):
                    osb = work.tile([128, 512], F32, tag="osb")
                    nc.vector.tensor_scalar_add(osb[:], ps_o[ct][:],
                                                bc_sb[:, ct:ct + 1])
                    nc.sync.dma_start(outT[ct * 128:(ct + 1) * 128, tsl],
                                      osb[:])

    nc.compile()
    return nc


def _get_nc():
    if "nc" not in _CACHE:
        _CACHE["nc"] = _build()
    return _CACHE["nc"]


def _make_in_maps(hidden_states, Wq, Wk, Wv, Wg, Wc, bc):
    hsT = np.ascontiguousarray(hidden_states.reshape(T, C).T,
                               dtype=np.float32)
    # pre-tile to [c*NT+n, 128, 512] so each projection DMA is one
    # contiguous 256KB burst
    hsT = np.ascontiguousarray(
        hsT.reshape(NC16, 128, NT, 512).transpose(0, 2, 1, 3)
        .reshape(NC16 * NT, 128, 512))
    tri = np.triu(np.ones((128, 128), dtype=np.float32))
    tri16 = np.triu(np.ones((16, 16), dtype=np.float32), k=1)
    ident = np.eye(128, dtype=np.float32)
    onesc = np.ones((128, 1), dtype=np.float32)
    onesr = np.ones((1, 128), dtype=np.float32)
    in_maps = []
    for i in range(N_CORES):
        in_maps.append({
            "hst": hsT,
            "wq": np.ascontiguousarray(
                Wq[:, i * 256:(i + 1) * 256], dtype=np.float32),
            "wk": np.ascontiguousarray(
                Wk[:, i * 128:(i + 1) * 128], dtype=np.float32),
            "wv": np.ascontiguousarray(
                Wv[:, i * 128:(i + 1) * 128], dtype=np.float32),
            "wg": np.ascontiguousarray(Wg[:, i:i + 1], dtype=np.float32),
            "wcs": np.ascontiguousarray(
                Wc[:, i * 256:(i + 1) * 256]).astype(ml_dtypes.bfloat16),
            "bcs": np.ascontiguousarray(
                bc[i * 256:(i + 1) * 256].reshape(2, 128).T,
                dtype=np.float32),
            "tri": tri,
            "tri16": tri16,
            "ident": ident,
            "onesc": onesc,
            "onesr": onesr,
        })
    return in_maps


def _run(in_maps, trace=False):
    nc = _get_nc()
    kw = {"tmpdir": "/tmp/trace_out"} if trace else {}
    res = run_bass_kernel_spmd(nc, in_maps, list(range(N_CORES)),
                               trace=trace, **kw)
    out = np.empty((T, C), dtype=np.float32)
    for i in range(N_CORES):
        out[:, i * 256:(i + 1) * 256] = res.results[i]["outT"].T
    return out.reshape(1, T, C), res


def kernel(hidden_states, Wq, Wk, Wv, Wg, Wc, bc):
    in_maps = _make_in_maps(np.asarray(hidden_states), np.asarray(Wq),
                            np.asarray(Wk), np.asarray(Wv), np.asarray(Wg),
                            np.asarray(Wc), np.asarray(bc))
    out, _ = _run(in_maps)
    return out
